# revision 2
# baseline (speedup 1.0000x reference)
"""MemNN layer kernel for 8 Trainium2 NeuronCores — v2.

Strategy (batch-sharded memory path, vocab-sharded projection):
- f16 megatable row v = [A0|A1|A2|A3][v] (512 f16 = 1KB). Per-core used-row
  compaction keeps every gather index < 17920 (int16-safe).
- 7 "supergroups" x 128 lanes: lanes 0-799 = (batch,story) slots, 800-815 =
  per-batch query-token lanes, 816-895 dummy (row 0 = zeros). Index order
  j = t_local*128 + lane, so a lane's 20 tokens stack on one partition in
  non-transpose gather layout [128, 20, 512].
- 14 non-transpose dma_gathers (1280 idx, single_packet=False) round-robin
  over 4 SWDGE queues: descgen runs concurrently per queue (~2-3 ns/idx
  aggregate), hidden under the ~51us gather DMA.
- Token sums: DVE pairwise-add tree (all-contiguous f16 tensor_tensor ops:
  20->10->5->(2+2)->...+q[4]), ~5.4us/supergroup, pipelined behind gathers.
- Embd-major flip: per supergroup, 4 PE transpose-matmuls (f16 lhsT x I128)
  into one PSUM bank -> strided copy -> S_em [128e, 4 tables, 896 lanes] f32.
  u0 = S_em[:, 0, 800:816].
- Hops: baseline embd-major code (logits via ones-matmul partition reduce,
  softmax on [1,800], p-broadcast via e0 matmul, weighted c-sum via DVE).
- Projection vocab-sharded: u -> DRAM -> CC AllGather (64KB fp32, ~10us) ->
  u_all16 [128e, 128b] lhsT; rhs = per-core a3t slice [128, 6256] f16;
  13 matmuls [128b, 512v] -> f16 ocache -> one 1.6MB store; host converts
  f32 and concatenates vocab slices.
"""

import numpy as np

HOPS = 3
VOCAB = 50000
EMBD = 128
BS = 128
STORY = 50
SENT = 20
QLEN = 20
NCORES = 8
BSH = BS // NCORES          # 16 batches per core
SLOTS = BSH * STORY         # 800 (b, s) story slots per core
NSG = 7                     # supergroups of 128 lanes
LANES = NSG * 128           # 896 (800 story + 16 query + 80 dummy)
NROWS = VOCAB + 1           # megatable rows (row layout after compaction)
NIDX = 6 * 2560 + 1024      # sg6: 48 lanes x 20 tok lane-major + 64 pad, transpose
VPAD = 50048                # vocab padded to 8*6256
VSH = VPAD // NCORES        # 6256 vocab rows per core
NVC = VSH // 512            # 12 full 512-chunks + remainder
_cache = {}
_last_res = None


def _wrap_idxs(lst):
    """int16 gather index layout: [128, n/16]; position i -> [i%16, i//16], tiled 8x."""
    a = np.asarray(lst).astype(np.int16).reshape(-1, 16).T.copy()
    return np.tile(a, (8, 1))


def _mk_ap(base_ap, dims, extra_offset_elems=0):
    import concourse.bass as bass
    ap = [tuple(base_ap.ap[0])] + [tuple(d) for d in dims]
    return bass.AP(base_ap.tensor, base_ap.offset + extra_offset_elems, ap)


def _build(dbg=False):
    import concourse.tile as tile
    from concourse import bacc, mybir

    f32 = mybir.dt.float32
    f16 = mybir.dt.float16
    i16 = mybir.dt.int16

    nc = bacc.Bacc("TRN2", target_bir_lowering=False, debug=False,
                   num_swdge_queues=4, num_devices=NCORES)

    mega = nc.dram_tensor("mega", [NROWS, 512], f16, kind="ExternalInput")
    a3t = nc.dram_tensor("a3t", [128, VSH], f16, kind="ExternalInput")
    idxs = nc.dram_tensor("idxs", [128, NIDX // 16], i16, kind="ExternalInput")
    tat = nc.dram_tensor("tat", [128, STORY], f32, kind="ExternalInput")
    tct = nc.dram_tensor("tct", [128, STORY], f32, kind="ExternalInput")
    ident = nc.dram_tensor("ident", [128, 128], f16, kind="ExternalInput")
    outp = nc.dram_tensor("outp", [128, VSH], f16, kind="ExternalOutput")
    if dbg:
        dbg_S = nc.dram_tensor("dbg_S", [128, 4 * LANES], f32, kind="ExternalOutput")
        dbg_u = nc.dram_tensor("dbg_u", [128, BS], f32, kind="ExternalOutput")
        dbg_G = nc.dram_tensor("dbg_G", [128, SENT * 512], f16, kind="ExternalOutput")
    warm_in = nc.dram_tensor("warm_in", [128, 2], f32, kind="Internal")
    warm_out = nc.dram_tensor("warm_out", [NCORES, 128, 2], f32,
                              kind="Internal", addr_space="Shared")
    u_loc = nc.dram_tensor("u_loc", [128, 2 * BSH], f16, kind="Internal")
    uall_loc = nc.dram_tensor("uall_loc", [NCORES, 128, 2 * BSH], f16,
                              kind="Internal", addr_space="Shared")

    with tile.TileContext(nc) as tc:
        with (
            tc.tile_pool(name="consts", bufs=1) as cpool,
            tc.tile_pool(name="gpool", bufs=5) as gpool,
            tc.tile_pool(name="tpool", bufs=2) as tpool,
            tc.tile_pool(name="psum", bufs=2, space="PSUM") as ppool,
        ):
            # ---- constants / small loads
            t_idx = cpool.tile([128, NIDX // 16], i16, tag="idx")
            nc.sync.dma_start(t_idx[:], idxs[:])
            t_tat = cpool.tile([128, STORY], f32, tag="tat")
            nc.sync.dma_start(t_tat[:], tat[:])
            t_tct = cpool.tile([128, STORY], f32, tag="tct")
            nc.sync.dma_start(t_tct[:], tct[:])
            t_id = cpool.tile([128, 128], f16, tag="ident")
            nc.sync.dma_start(t_id[:], ident[:])
            t_idf = cpool.tile([128, 128], f32, tag="identf")
            nc.vector.tensor_copy(t_idf[:], t_id[:])
            ones_col = cpool.tile([128, 1], f32, tag="ones_col")
            nc.vector.memset(ones_col[:], 1.0)
            e0row = cpool.tile([128, 128], f16, tag="e0row")
            nc.vector.memset(e0row[:], 0.0)
            nc.vector.memset(e0row[0:1, :], 1.0)

            S = cpool.tile([128, 4, LANES], f32, tag="S")  # embd-major sums

            # CC pipeline warmup: tiny AllGather early, hides under gathers
            warm_t = cpool.tile([128, 2], f32, tag="warm")
            nc.vector.memset(warm_t[:], 0.0)
            nc.sync.dma_start(warm_in[:], warm_t[:])
            nc.gpsimd.collective_compute(
                "AllGather", mybir.AluOpType.bypass,
                replica_groups=[list(range(NCORES))],
                ins=[warm_in[:].opt()], outs=[warm_out[:].opt()])

            # ---- gather + token-sum + transpose, per supergroup
            lp = nc.allow_low_precision(reason="f16 pairwise token sums")
            lp.__enter__()
            NSG_NT = NSG - 1  # sg6 handled via one transpose gather
            Gt = cpool.tile([128, 4, 1024], f16, tag="Gt")
            nc.gpsimd.dma_gather(
                Gt[:], mega[:], t_idx[:, NSG_NT * 160:NSG_NT * 160 + 64],
                1024, 1024, 512, transpose=True, single_packet=False,
                queue_num=3)
            S6 = _mk_ap(S[:], [(LANES, 4), (1, 48)], NSG_NT * 128)
            red6 = _mk_ap(Gt[:], [(1024, 4), (SENT, 48), (1, SENT)])
            nc.vector.tensor_reduce(S6, red6, mybir.AxisListType.X,
                                    mybir.AluOpType.add)
            for g in range(NSG_NT):
                Gh = []
                for h in range(2):
                    Gp = gpool.tile([128, 10, 512], f16, tag=f"G{h}")
                    cs = slice((g * 2 + h) * 80, (g * 2 + h + 1) * 80)
                    nc.gpsimd.dma_gather(
                        Gp[:], mega[:], t_idx[:, cs],
                        1280, 1280, 512, transpose=False, single_packet=False,
                        queue_num=(g * 2 + h) % 3)
                    Gh.append(Gp)

                eng = nc.gpsimd if g >= 4 else nc.vector
                G = Gh[0]
                nc.vector.tensor_add(G[:], G[:], Gh[1][:])
                t_q = tpool.tile([128, 5, 512], f32, tag="q")
                nc.vector.tensor_add(t_q[:], G[:, 0:5, :], G[:, 5:10, :])
                t_r1 = tpool.tile([128, 2, 512], f32, tag="r1")
                eng.tensor_add(t_r1[:], t_q[:, 0:2, :], t_q[:, 2:4, :])
                t_r2 = tpool.tile([128, 512], f32, tag="r2")
                eng.tensor_add(t_r2[:], t_r1[:, 0, :], t_r1[:, 1, :])
                t_r3 = tpool.tile([128, 512], f32, tag="r3")
                eng.tensor_add(t_r3[:], t_r2[:], t_q[:, 4, :])
                # transpose 4 slices into one PSUM bank, then strided copy out
                pt = ppool.tile([128, 512], f32, tag="pt", space="PSUM")
                for k in range(4):
                    nc.tensor.matmul(pt[:, k * 128:(k + 1) * 128],
                                     lhsT=t_r3[:, k * 128:(k + 1) * 128],
                                     rhs=t_idf[:], start=True, stop=True)
                dst = _mk_ap(S[:], [(LANES, 4), (1, 128)], g * 128)
                nc.scalar.activation(dst, pt[:], mybir.ActivationFunctionType.Copy)
            lp.__exit__(None, None, None)
            t_a3 = cpool.tile([128, VSH], f16, tag="a3")
            nc.sync.dma_start(t_a3[:], a3t[:])

            if dbg:
                nc.sync.dma_start(dbg_S[:], _mk_ap(S[:], [(1, 4 * LANES)]))

            # ---- pre-bias: m0 = S0+TA, c3 = S3+TC in place; c1, c2 to tiles (gpsimd)
            ta_b0 = _mk_ap(t_tat[:], [(0, BSH), (1, STORY)])
            tc_b0 = _mk_ap(t_tct[:], [(0, BSH), (1, STORY)])
            c12 = cpool.tile([128, 2, SLOTS], f32, tag="c12")
            def sv(k):
                return _mk_ap(S[:], [(STORY, BSH), (1, STORY)], k * LANES)
            nc.gpsimd.tensor_add(sv(0), sv(0), ta_b0)
            nc.gpsimd.tensor_add(sv(3), sv(3), tc_b0)
            for kk in (1, 2):
                cdst = _mk_ap(c12[:], [(STORY, BSH), (1, STORY)], (kk - 1) * SLOTS)
                nc.gpsimd.tensor_add(cdst, sv(kk), tc_b0)

            # ---- hops (embd-major, baseline structure, scale=1)
            u = cpool.tile([128, BSH], f32, tag="u")
            nc.vector.tensor_copy(u[:], S[:, 0, SLOTS:SLOTS + BSH])

            t0 = cpool.tile([128, BSH, STORY], f32, tag="t0")
            pe_sb = cpool.tile([128, BSH, STORY], f16, tag="pe_sb")
            nc.vector.memset(pe_sb[:], 0.0)
            lg = cpool.tile([1, BSH, STORY], f32, tag="lg")
            red = cpool.tile([1, BSH], f32, tag="red")
            red2 = cpool.tile([1, BSH], f32, tag="red2")
            red_u = cpool.tile([128, BSH], f32, tag="redu")

            def smv(k, off=0, nb=BSH):
                return _mk_ap(S[:], [(STORY, nb), (1, STORY)], k * LANES + off * STORY)

            def t0v(off=0, nb=BSH):
                return _mk_ap(t0[:], [(STORY, nb), (1, STORY)], off * STORY)

            def t0f(off, n):
                return _mk_ap(t0[:], [(1, n)], off)

            ta_b = _mk_ap(t_tat[:], [(0, BSH), (1, STORY)])
            tc_bh = _mk_ap(t_tct[:], [(0, BSH // 2), (1, STORY)])
            u_b = _mk_ap(u[:], [(1, BSH), (0, STORY)])
            HB = SLOTS // 2  # 400

            for k in range(HOPS):
                HB2 = BSH // 2
                u_bh = [_mk_ap(u[:], [(1, HB2), (0, STORY)], hh * HB2)
                        for hh in range(2)]
                if k == 0:
                    nc.vector.tensor_mul(t0v(0, HB2), smv(0, 0, HB2), u_bh[0])
                    nc.gpsimd.tensor_mul(t0v(HB2, HB2), smv(0, HB2, HB2), u_bh[1])
                else:
                    ta_bh = _mk_ap(t_tat[:], [(0, HB2), (1, STORY)])
                    for hh, e2 in ((0, nc.vector), (1, nc.gpsimd)):
                        e2.tensor_add(t0v(hh * HB2, HB2), smv(k, hh * HB2, HB2), ta_bh)
                        e2.tensor_mul(t0v(hh * HB2, HB2), t0v(hh * HB2, HB2), u_bh[hh])
                for hh in range(2):
                    pl = ppool.tile([1, HB], f32, tag="pl", space="PSUM")
                    nc.tensor.matmul(
                        pl[:], lhsT=ones_col[:], rhs=t0f(hh * HB, HB),
                        start=True, stop=True)
                    nc.scalar.activation(
                        _mk_ap(lg[:], [(1, HB)], hh * HB), pl[:],
                        mybir.ActivationFunctionType.Exp)
                nc.vector.tensor_reduce(red2[:], lg[:], mybir.AxisListType.X, mybir.AluOpType.add)
                nc.vector.reciprocal(red2[:], red2[:])
                red2_b = _mk_ap(red2[:], [(1, BSH), (0, STORY)])
                nc.vector.tensor_mul(pe_sb[0:1, :, :], lg[:], red2_b)
                for hh in range(2):
                    pb = ppool.tile([128, HB], f32, tag="pb", space="PSUM")
                    nc.tensor.matmul(
                        pb[:], lhsT=e0row[:],
                        rhs=_mk_ap(pe_sb[:], [(1, HB)], hh * HB),
                        start=True, stop=True)
                    pb3 = _mk_ap(pb[:], [(STORY, BSH // 2), (1, STORY)])
                    if k == 2:
                        csrc = smv(3, hh * (BSH // 2), BSH // 2)
                    else:
                        csrc = _mk_ap(c12[:], [(STORY, BSH // 2), (1, STORY)],
                                      k * SLOTS + hh * (BSH // 2) * STORY)
                    nc.vector.tensor_mul(
                        t0v(hh * (BSH // 2), BSH // 2), csrc, pb3)
                nc.vector.tensor_reduce(red_u[:], t0v(), mybir.AxisListType.X, mybir.AluOpType.add)
                nc.vector.tensor_add(u[:], u[:], red_u[:])

            # ---- u all-gather (packed [u16|du16] f16 CC) then projection
            import concourse.bass as bass
            upk = cpool.tile([128, 2 * BSH], f16, tag="upk")
            nc.vector.tensor_copy(upk[:, 0:BSH], u[:])
            udr = cpool.tile([128, BSH], f32, tag="udr")
            nc.vector.tensor_sub(udr[:], u[:], upk[:, 0:BSH])
            nc.vector.tensor_copy(upk[:, BSH:2 * BSH], udr[:])
            nc.sync.dma_start(u_loc[:], upk[:])
            nc.gpsimd.collective_compute(
                "AllGather", mybir.AluOpType.bypass,
                replica_groups=[list(range(NCORES))],
                ins=[u_loc[:].opt()], outs=[uall_loc[:].opt()])
            u_all16 = cpool.tile([128, BS], f16, tag="uall16")
            du16 = cpool.tile([128, BS], f16, tag="du16")
            nc.sync.dma_start(u_all16[:], bass.AP(
                uall_loc, 0, [(2 * BSH, 128), (128 * 2 * BSH, NCORES), (1, BSH)]))
            nc.sync.dma_start(du16[:], bass.AP(
                uall_loc, BSH, [(2 * BSH, 128), (128 * 2 * BSH, NCORES), (1, BSH)]))
            if dbg:
                u_dbg = cpool.tile([128, BS], f32, tag="u_dbg")
                nc.vector.tensor_add(u_dbg[:], u_all16[:], du16[:])
                nc.sync.dma_start(dbg_u[:], u_dbg[:])

            ocache = cpool.tile([128, VSH], f16, tag="ocache")
            nchunks = (VSH + 511) // 512
            for j in range(nchunks):
                w = min(512, VSH - j * 512)
                po = ppool.tile([128, 512], f32, tag="po", space="PSUM")
                nc.tensor.matmul(po[:, :w], lhsT=u_all16[:],
                                 rhs=t_a3[:, j * 512:j * 512 + w],
                                 start=True, stop=False)
                nc.tensor.matmul(po[:, :w], lhsT=du16[:],
                                 rhs=t_a3[:, j * 512:j * 512 + w],
                                 start=False, stop=True)
                nc.scalar.activation(ocache[:, j * 512:j * 512 + w], po[:, :w], mybir.ActivationFunctionType.Copy)
                if j % 4 == 3 or j == nchunks - 1:
                    lo = (j // 4) * 4 * 512
                    hi = j * 512 + w
                    nc.sync.dma_start(outp[:, lo:hi], ocache[:, lo:hi])

    nc.compile()
    return nc


def _prep_inputs(x, q, A, TA, TC):
    x = np.asarray(x).astype(np.int64)
    q = np.asarray(q).astype(np.int64)
    A = np.asarray(A, dtype=np.float32)
    TA = np.asarray(TA, dtype=np.float32)
    TC = np.asarray(TC, dtype=np.float32)

    megaF = np.zeros((NROWS, 512), dtype=np.float16)
    for k in range(4):
        megaF[:VOCAB, k * 128:(k + 1) * 128] = A[k].astype(np.float16)

    a3_full = np.zeros((128, VPAD), dtype=np.float16)
    a3_full[:, :VOCAB] = A[3].astype(np.float16).T

    tat = np.ascontiguousarray(TA[0].T)  # [128, 50]
    tct = np.ascontiguousarray(TC[0].T)
    ident = np.eye(128, dtype=np.float16)

    in_maps = []
    for c in range(NCORES):
        xs = x[c * BSH:(c + 1) * BSH]                    # [16, 50, 20]
        qs = q[c * BSH:(c + 1) * BSH]                    # [16, 20]
        used = np.unique(np.concatenate([[0], xs.reshape(-1), qs.reshape(-1)]))
        nu = used.shape[0]
        order = np.empty(NROWS, dtype=np.int64)
        order[:nu] = used
        mask = np.ones(NROWS, dtype=bool)
        mask[used] = False
        order[nu:] = np.nonzero(mask)[0]
        pos = np.empty(NROWS, dtype=np.int64)
        pos[order] = np.arange(NROWS)
        mega_c = megaF[order]

        # lane table: lane L -> 20 token rows (compacted)
        lane_tok = np.zeros((SLOTS + BSH, SENT), dtype=np.int64)
        lane_tok[:SLOTS] = pos[xs.reshape(SLOTS, SENT)]
        lane_tok[SLOTS:SLOTS + BSH] = pos[qs]
        # sgs 0-5: j = t_local*128 + lane_local (non-transpose);
        # sg6 (lanes 768-815): j = lane_local*20 + t, padded to 1024 (transpose)
        idx = np.zeros(NIDX, dtype=np.int64)
        p_ = 0
        for g in range(6):
            lanes = lane_tok[g * 128:(g + 1) * 128]       # [128, 20]
            for h in range(2):
                blk = lanes[:, 10 * h:10 * h + 10].T      # [10 tok, 128 lane]
                idx[p_:p_ + 1280] = blk.reshape(-1)
                p_ += 1280
        idx[p_:p_ + 960] = lane_tok[768:816].reshape(-1)  # lane-major
        p_ += 1024
        in_maps.append({
            "mega": mega_c, "a3t": a3_full[:, c * VSH:(c + 1) * VSH],
            "idxs": _wrap_idxs(idx), "tat": tat, "tct": tct, "ident": ident,
        })
    return in_maps


def kernel(x, q, A, TA, TC):
    import os
    from concourse.bass_utils import run_bass_kernel_spmd

    in_maps = _prep_inputs(x, q, A, TA, TC)
    dbg = bool(int(os.environ.get("MEMNN_DEBUG", "0")))
    if _cache.get("dbg") != dbg:
        _cache["nc"] = _build(dbg)
        _cache["dbg"] = dbg
    nc = _cache["nc"]
    trace = bool(int(os.environ.get("MEMNN_TRACE", "0")))
    res = run_bass_kernel_spmd(nc, in_maps, list(range(NCORES)), trace=trace)
    if trace:
        _cache["exec_time_ns"] = res.exec_time_ns
        _cache["mean_exec_time_ns"] = res.mean_exec_time_ns
        _cache["results"] = res
    global _last_res
    _last_res = res

    out = np.empty((BS, VPAD), dtype=np.float32)
    for c in range(NCORES):
        out[:, c * VSH:(c + 1) * VSH] = res.results[c]["outp"].astype(np.float32)
    return out[:, :VOCAB]
